# revision 8
# baseline (speedup 1.0000x reference)
"""RWKV WKV attention kernel for TRN2 (Bass/Tile), batch-parallel over 8 cores.

v3: host-transposed bf16 x (leading zero halo column), bf16 matmuls,
fused output projection (no DRAM round-trip), scans split across
Vector and GpSimd, sigmoid via the Exp table, den via one stt.

Per core (one batch element):
  chunk loop over T in TC=512 steps:
    DMA xT halo tile [128, TC+1] per d-group j (halo = col t0-1).
    diff/mix (bf16, DVE) -> k,v,r projections (bf16 matmul, fp32 PSUM).
    ek = exp(k); a' = ek*v; scans sa (DVE) / sb (GpSimd);
    num = a'*e^u + sa; den = ek*e^u + sb; den2 = (1+e^-r)*den;
    rw = num * recip(den2) (bf16) -> out = rw^T @ Wo^T (fused, per chunk).

Host-packed weights [128, 8*1024] bf16: arr[p, j*1024+e] = W[e, j*128+p].
cv [128, 40] fp32 (col j of group g = channels j*128..j*128+127):
  0-7 mk, 8-15 mv, 16-23 mr, 24-31 ew=exp(-exp(time_decay)), 32-39 eu=e^u.
"""
import sys
for p in ("/opt/trn_rl_repo",):
    if p not in sys.path:
        sys.path.insert(0, p)

import numpy as np
from contextlib import ExitStack

import concourse.bass as bass
import concourse.tile as tile
from concourse import bacc, mybir

dt = mybir.dt
AF = mybir.ActivationFunctionType
OP = mybir.AluOpType

D = 1024
NJ = D // 128  # 8 channel chunks


def build(nc, T=4096, TC=512):
    nch = T // TC
    NTS = TC // 128

    XT = nc.dram_tensor("xt", [D, T + 1], dt.bfloat16, kind="ExternalInput").ap()
    WK = nc.dram_tensor("wk", [128, NJ * D], dt.bfloat16, kind="ExternalInput").ap()
    WV = nc.dram_tensor("wv", [128, NJ * D], dt.bfloat16, kind="ExternalInput").ap()
    WR = nc.dram_tensor("wr", [128, NJ * D], dt.bfloat16, kind="ExternalInput").ap()
    WO = nc.dram_tensor("wo", [128, NJ * D], dt.bfloat16, kind="ExternalInput").ap()
    CV = nc.dram_tensor("cv", [128, 40], dt.float32, kind="ExternalInput").ap()
    O = nc.dram_tensor("o", [T, D], dt.float32, kind="ExternalOutput").ap()

    with tile.TileContext(nc) as tc, ExitStack() as ctx:
        wpool = ctx.enter_context(tc.tile_pool(name="wpool", bufs=1))
        xp = ctx.enter_context(tc.tile_pool(name="xp", bufs=2 * NJ + 2))
        dfp = ctx.enter_context(tc.tile_pool(name="dfp", bufs=2))
        mkp = ctx.enter_context(tc.tile_pool(name="mkp", bufs=2 * NJ))
        mvp = ctx.enter_context(tc.tile_pool(name="mvp", bufs=2 * NJ))
        mrp = ctx.enter_context(tc.tile_pool(name="mrp", bufs=2 * NJ))
        kp = ctx.enter_context(tc.tile_pool(name="kp", bufs=2, space="PSUM"))
        vp = ctx.enter_context(tc.tile_pool(name="vp", bufs=2, space="PSUM"))
        rp = ctx.enter_context(tc.tile_pool(name="rp", bufs=2, space="PSUM"))
        outp = ctx.enter_context(tc.tile_pool(name="outp", bufs=2, space="PSUM"))
        ekp = ctx.enter_context(tc.tile_pool(name="ekp", bufs=3))
        app = ctx.enter_context(tc.tile_pool(name="app", bufs=3))
        onep = ctx.enter_context(tc.tile_pool(name="onep", bufs=3))
        vsp = ctx.enter_context(tc.tile_pool(name="vsp", bufs=2))
        sap = ctx.enter_context(tc.tile_pool(name="sap", bufs=2))
        sbp = ctx.enter_context(tc.tile_pool(name="sbp", bufs=2))
        nump = ctx.enter_context(tc.tile_pool(name="nump", bufs=2))
        denp = ctx.enter_context(tc.tile_pool(name="denp", bufs=2))
        dn2p = ctx.enter_context(tc.tile_pool(name="dn2p", bufs=2))
        erp = ctx.enter_context(tc.tile_pool(name="erp", bufs=2))
        rwp = ctx.enter_context(tc.tile_pool(name="rwp", bufs=2 * NJ))
        ocp = ctx.enter_context(tc.tile_pool(name="ocp", bufs=4))
        stp = ctx.enter_context(tc.tile_pool(name="stp", bufs=1))

        wk_t = wpool.tile([128, NJ * D], dt.bfloat16, tag="wk")
        nc.sync.dma_start(wk_t[:], WK)
        wv_t = wpool.tile([128, NJ * D], dt.bfloat16, tag="wv")
        nc.sync.dma_start(wv_t[:], WV)
        wr_t = wpool.tile([128, NJ * D], dt.bfloat16, tag="wr")
        nc.sync.dma_start(wr_t[:], WR)
        cv = wpool.tile([128, 40], dt.float32, tag="cv")
        nc.sync.dma_start(cv[:], CV)
        wo_t = wpool.tile([128, NJ * D], dt.bfloat16, tag="wo")
        nc.sync.dma_start(wo_t[:], WO)

        def states(prefix):
            ts_ = []
            for e in range(NJ):
                t = stp.tile([128, 1], dt.float32, tag=f"{prefix}{e}")
                nc.vector.memset(t[:], 0.0)
                ts_.append(t)
            return ts_

        ekst = states("ekst")   # ek halo carry (scalar engine)
        ast = states("ast")     # a' halo carry (vector)
        alst = states("alst")   # sa scan carry (vector)
        best = states("best")   # sb scan carry (gpsimd)

        def load_x(c):
            t0 = c * TC
            xh = []
            for j in range(NJ):
                x_ = xp.tile([128, TC + 1], dt.bfloat16, tag="xh")
                nc.sync.dma_start(x_[:], XT[j * 128:(j + 1) * 128, t0:t0 + TC + 1])
                xh.append(x_)
            return xh

        def mix_stage(xh):
            """diff + time-mix for k/v/r, all bf16 on DVE."""
            mk_l, mv_l, mr_l = [], [], []
            for j in range(NJ):
                d_ = dfp.tile([128, TC], dt.bfloat16, tag="df")
                nc.vector.tensor_tensor(d_[:], xh[j][:, 1:TC + 1], xh[j][:, 0:TC],
                                        OP.subtract)
                for pi, (pool, lst) in enumerate(((mkp, mk_l), (mvp, mv_l),
                                                  (mrp, mr_l))):
                    m_ = pool.tile([128, TC], dt.bfloat16, tag=f"m{pi}")
                    nc.vector.scalar_tensor_tensor(
                        m_[:], d_[:], cv[:, pi * 8 + j:pi * 8 + j + 1],
                        xh[j][:, 0:TC], OP.mult, OP.add)
                    lst.append(m_)
            return mk_l, mv_l, mr_l

        def chunk_body(c, mixes):
            """k/v/r matmuls + WKV chain for chunk c; returns rw tiles.

            Front half (per e): matmuls, scalar exp's, gpsimd a'.
            Back half (per e, skewed by one): vector scans/num/den/recip,
            gpsimd den2/rw. The skew keeps a'(e) ready before scan_a(e).
            """
            mk_l, mv_l, mr_l = mixes
            rws = []
            front = {}

            def front_half(e):
                acck = kp.tile([128, TC], dt.float32, tag="acck")
                for j in range(NJ):
                    nc.tensor.matmul(
                        acck[:], wk_t[:, j * D + e * 128: j * D + (e + 1) * 128],
                        mk_l[j][:], start=(j == 0), stop=(j == NJ - 1))
                accv = vp.tile([128, TC], dt.float32, tag="accv")
                for j in range(NJ):
                    nc.tensor.matmul(
                        accv[:], wv_t[:, j * D + e * 128: j * D + (e + 1) * 128],
                        mv_l[j][:], start=(j == 0), stop=(j == NJ - 1))
                accr = rp.tile([128, TC], dt.float32, tag="accr")
                for j in range(NJ):
                    nc.tensor.matmul(
                        accr[:], wr_t[:, j * D + e * 128: j * D + (e + 1) * 128],
                        mr_l[j][:], start=(j == 0), stop=(j == NJ - 1))

                # scalar: ek = exp(k) with halo, er = exp(-r), oner = 1+er
                ek = ekp.tile([128, TC + 1], dt.float32, tag="ek")
                nc.scalar.copy(ek[:, 0:1], ekst[e][:])
                nc.scalar.activation(ek[:, 1:TC + 1], acck[:], AF.Exp)
                nc.scalar.copy(ekst[e][:], ek[:, TC:TC + 1])
                er = erp.tile([128, TC], dt.float32, tag="er")
                nc.scalar.activation(er[:], accr[:], AF.Exp, scale=-1.0)
                oner = onep.tile([128, TC], dt.float32, tag="oner")
                nc.scalar.activation(oner[:], er[:], AF.Copy, bias=1.0)
                vsb = vsp.tile([128, TC], dt.float32, tag="vsb")
                nc.scalar.copy(vsb[:], accv[:])

                # gpsimd: a' = ek*v with halo (gpsimd cannot touch PSUM)
                a_ = app.tile([128, TC + 1], dt.float32, tag="a")
                nc.scalar.copy(a_[:, 0:1], ast[e][:])
                nc.gpsimd.tensor_tensor(a_[:, 1:TC + 1], ek[:, 1:TC + 1], vsb[:],
                                        OP.mult)
                nc.scalar.copy(ast[e][:], a_[:, TC:TC + 1])
                front[e] = (ek, a_, oner)

            def back_half(e):
                ek, a_, oner = front.pop(e)
                H = TC // 2
                ewb = cv[:, 24 + e: 25 + e].broadcast_to([128, H])
                # scans split into two chained 256-halves (DVE's >256 path
                # is disproportionately slow: 2030ns vs 2x600ns)
                sa = sap.tile([128, TC], dt.float32, tag="sa")
                nc.vector.tensor_tensor_scan(sa[:, 0:H], ewb, a_[:, 0:H],
                                             alst[e][:], OP.mult, OP.add)
                nc.vector.tensor_tensor_scan(sa[:, H:TC], ewb, a_[:, H:TC],
                                             sa[:, H - 1:H], OP.mult, OP.add)
                nc.scalar.copy(alst[e][:], sa[:, TC - 1:TC])
                sb = sbp.tile([128, TC], dt.float32, tag="sb")
                nc.vector.tensor_tensor_scan(sb[:, 0:H], ewb, ek[:, 0:H],
                                             best[e][:], OP.mult, OP.add)
                nc.vector.tensor_tensor_scan(sb[:, H:TC], ewb, ek[:, H:TC],
                                             sb[:, H - 1:H], OP.mult, OP.add)
                nc.scalar.copy(best[e][:], sb[:, TC - 1:TC])

                eu = cv[:, 32 + e: 33 + e]
                num = nump.tile([128, TC], dt.float32, tag="num")
                nc.vector.scalar_tensor_tensor(num[:], a_[:, 1:TC + 1], eu, sa[:],
                                               OP.mult, OP.add)
                den = denp.tile([128, TC], dt.float32, tag="den")
                nc.vector.scalar_tensor_tensor(den[:], ek[:, 1:TC + 1], eu, sb[:],
                                               OP.mult, OP.add)
                # gpsimd: den2 = den*(1+er); vector: recip; gpsimd: rw
                dn2 = dn2p.tile([128, TC], dt.float32, tag="dn2")
                nc.gpsimd.tensor_tensor(dn2[:], den[:], oner[:], OP.mult)
                nc.vector.reciprocal_approx_fast(dn2[:], dn2[:])
                rw = rwp.tile([128, TC], dt.bfloat16, tag="rw")
                nc.gpsimd.tensor_tensor(rw[:], num[:], dn2[:], OP.mult)
                rws.append(rw)

            for e in range(NJ):
                front_half(e)
                if e > 0:
                    back_half(e - 1)
            back_half(NJ - 1)
            return rws

        def out_stage(c, rws):
            """out = rw^T @ Wo^T for chunk c, straight to DRAM."""
            t0 = c * TC
            for ts_ in range(NTS):
                for eh in range(2):
                    op = outp.tile([128, 512], dt.float32, tag="op")
                    for j in range(NJ):
                        nc.tensor.matmul(
                            op[:], rws[j][:, ts_ * 128:(ts_ + 1) * 128],
                            wo_t[:, j * D + eh * 512: j * D + (eh + 1) * 512],
                            start=(j == 0), stop=(j == NJ - 1))
                    oc = ocp.tile([128, 512], dt.float32, tag="oc")
                    nc.scalar.copy(oc[:], op[:])
                    nc.scalar.dma_start(
                        O[t0 + ts_ * 128: t0 + (ts_ + 1) * 128,
                          eh * 512:(eh + 1) * 512], oc[:])

        # ---- pipelined chunk loop ----
        xh0 = load_x(0)
        xh1 = load_x(1)
        mixes = mix_stage(xh0)
        xh_next = xh1
        rws_prev = None
        for c in range(nch):
            if c + 2 < nch:
                xh_fut = load_x(c + 2)
            else:
                xh_fut = None
            if c + 1 < nch:
                mixes_next = mix_stage(xh_next)
            rws = chunk_body(c, mixes)
            if rws_prev is not None:
                out_stage(c - 1, rws_prev)
            rws_prev = rws
            if c + 1 < nch:
                mixes = mixes_next
                xh_next = xh_fut
        out_stage(nch - 1, rws_prev)


def pack_inputs(x_slice, time_decay, time_first, time_mix_k, time_mix_v,
                time_mix_r, Wk, Wv, Wr, Wo):
    """Host-side packing for one core. x_slice: [T, D] fp32."""
    import ml_dtypes
    bf16 = ml_dtypes.bfloat16

    def packw(W):
        return np.ascontiguousarray(
            W.T.reshape(NJ, 128, D).transpose(1, 0, 2).reshape(128, NJ * D)
        ).astype(bf16)

    def packv(v):
        return np.ascontiguousarray(v.reshape(NJ, 128).T).astype(np.float32)

    T = x_slice.shape[0]
    xt = np.zeros((D, T + 1), dtype=bf16)
    xt[:, 1:] = x_slice.T.astype(bf16)

    mk = time_mix_k.reshape(D).astype(np.float32)
    mv = time_mix_v.reshape(D).astype(np.float32)
    mr = time_mix_r.reshape(D).astype(np.float32)
    ew = np.exp(-np.exp(time_decay.astype(np.float32))).astype(np.float32)
    eu = np.exp(time_first.astype(np.float32).reshape(D)).astype(np.float32)
    cv = np.concatenate([
        packv(mk), packv(mv), packv(mr), packv(ew), packv(eu)],
        axis=1).astype(np.float32)
    return {
        "xt": np.ascontiguousarray(xt),
        "wk": packw(Wk), "wv": packw(Wv), "wr": packw(Wr), "wo": packw(Wo),
        "cv": cv,
    }


# ---------------------------------------------------------------------------
# Harness entry point: full inputs in, full output out, 8-way batch-parallel.
# ---------------------------------------------------------------------------
_CACHE = {}
_last_exec_time_ns = None


def _get_program(n_cores):
    key = ("prog", n_cores)
    if key not in _CACHE:
        nc = bacc.Bacc("TRN2", target_bir_lowering=False, debug=False,
                       num_devices=n_cores)
        build(nc, T=4096)
        nc.compile()
        _CACHE[key] = nc
    return _CACHE[key]


def kernel(x, time_decay, time_first, time_mix_k, time_mix_v, time_mix_r,
           Wk, Wv, Wr, Wo):
    """WKV attention: x [8, 4096, 1024] fp32 -> out [8, 4096, 1024] fp32.

    Shards batch across the 8 NeuronCores (one batch element per core).
    """
    global _last_exec_time_ns
    import os
    from concourse import bass_utils

    x = np.asarray(x, dtype=np.float32)
    B = x.shape[0]
    base = pack_inputs(x[0], np.asarray(time_decay), np.asarray(time_first),
                       np.asarray(time_mix_k), np.asarray(time_mix_v),
                       np.asarray(time_mix_r), np.asarray(Wk), np.asarray(Wv),
                       np.asarray(Wr), np.asarray(Wo))
    import ml_dtypes
    bf16 = ml_dtypes.bfloat16
    in_maps = []
    for b in range(B):
        m = dict(base)
        xt = np.zeros((D, x.shape[1] + 1), dtype=bf16)
        xt[:, 1:] = x[b].T.astype(bf16)
        m["xt"] = np.ascontiguousarray(xt)
        in_maps.append(m)

    nc = _get_program(B)
    trace = os.environ.get("WKV_TRACE", "0") == "1"
    r = bass_utils.run_bass_kernel_spmd(nc, in_maps, core_ids=list(range(B)),
                                        trace=trace)
    _last_exec_time_ns = r.exec_time_ns
    return np.stack([r.results[b]["o"] for b in range(B)]).astype(np.float32)


# revision 9
# speedup vs baseline: 1.0382x; 1.0382x over previous
"""RWKV WKV attention kernel for TRN2 (Bass/Tile), batch-parallel over 8 cores.

v3: host-transposed bf16 x (leading zero halo column), bf16 matmuls,
fused output projection (no DRAM round-trip), scans split across
Vector and GpSimd, sigmoid via the Exp table, den via one stt.

Per core (one batch element):
  chunk loop over T in TC=512 steps:
    DMA xT halo tile [128, TC+1] per d-group j (halo = col t0-1).
    diff/mix (bf16, DVE) -> k,v,r projections (bf16 matmul, fp32 PSUM).
    ek = exp(k); a' = ek*v; scans sa (DVE) / sb (GpSimd);
    num = a'*e^u + sa; den = ek*e^u + sb; den2 = (1+e^-r)*den;
    rw = num * recip(den2) (bf16) -> out = rw^T @ Wo^T (fused, per chunk).

Host-packed weights [128, 8*1024] bf16: arr[p, j*1024+e] = W[e, j*128+p].
cv [128, 40] fp32 (col j of group g = channels j*128..j*128+127):
  0-7 mk, 8-15 mv, 16-23 mr, 24-31 ew=exp(-exp(time_decay)), 32-39 eu=e^u.
"""
import sys
for p in ("/opt/trn_rl_repo",):
    if p not in sys.path:
        sys.path.insert(0, p)

import numpy as np
from contextlib import ExitStack

import concourse.bass as bass
import concourse.tile as tile
from concourse import bacc, mybir

dt = mybir.dt
AF = mybir.ActivationFunctionType
OP = mybir.AluOpType

D = 1024
NJ = D // 128  # 8 channel chunks


def build(nc, T=4096, TC=512):
    nch = T // TC
    NTS = TC // 128

    XT = nc.dram_tensor("xt", [D, T + 1], dt.bfloat16, kind="ExternalInput").ap()
    WK = nc.dram_tensor("wk", [128, NJ * D], dt.bfloat16, kind="ExternalInput").ap()
    WV = nc.dram_tensor("wv", [128, NJ * D], dt.bfloat16, kind="ExternalInput").ap()
    WR = nc.dram_tensor("wr", [128, NJ * D], dt.bfloat16, kind="ExternalInput").ap()
    WO = nc.dram_tensor("wo", [128, NJ * D], dt.bfloat16, kind="ExternalInput").ap()
    CV = nc.dram_tensor("cv", [128, 40], dt.float32, kind="ExternalInput").ap()
    O = nc.dram_tensor("o", [T, D], dt.float32, kind="ExternalOutput").ap()

    with tile.TileContext(nc) as tc, ExitStack() as ctx:
        wpool = ctx.enter_context(tc.tile_pool(name="wpool", bufs=1))
        xp = ctx.enter_context(tc.tile_pool(name="xp", bufs=2 * NJ + 2))
        dfp = ctx.enter_context(tc.tile_pool(name="dfp", bufs=2))
        mkp = ctx.enter_context(tc.tile_pool(name="mkp", bufs=2 * NJ))
        mvp = ctx.enter_context(tc.tile_pool(name="mvp", bufs=2 * NJ))
        mrp = ctx.enter_context(tc.tile_pool(name="mrp", bufs=2 * NJ))
        kp = ctx.enter_context(tc.tile_pool(name="kp", bufs=2, space="PSUM"))
        vp = ctx.enter_context(tc.tile_pool(name="vp", bufs=2, space="PSUM"))
        rp = ctx.enter_context(tc.tile_pool(name="rp", bufs=2, space="PSUM"))
        outp = ctx.enter_context(tc.tile_pool(name="outp", bufs=2, space="PSUM"))
        ekp = ctx.enter_context(tc.tile_pool(name="ekp", bufs=3))
        app = ctx.enter_context(tc.tile_pool(name="app", bufs=3))
        onep = ctx.enter_context(tc.tile_pool(name="onep", bufs=3))
        vsp = ctx.enter_context(tc.tile_pool(name="vsp", bufs=2))
        sap = ctx.enter_context(tc.tile_pool(name="sap", bufs=2))
        sbp = ctx.enter_context(tc.tile_pool(name="sbp", bufs=2))
        nump = ctx.enter_context(tc.tile_pool(name="nump", bufs=2))
        denp = ctx.enter_context(tc.tile_pool(name="denp", bufs=2))
        dn2p = ctx.enter_context(tc.tile_pool(name="dn2p", bufs=2))
        erp = ctx.enter_context(tc.tile_pool(name="erp", bufs=2))
        rwp = ctx.enter_context(tc.tile_pool(name="rwp", bufs=2 * NJ))
        ocp = ctx.enter_context(tc.tile_pool(name="ocp", bufs=4))
        stp = ctx.enter_context(tc.tile_pool(name="stp", bufs=1))

        wk_t = wpool.tile([128, NJ * D], dt.bfloat16, tag="wk")
        nc.sync.dma_start(wk_t[:], WK)
        wv_t = wpool.tile([128, NJ * D], dt.bfloat16, tag="wv")
        nc.sync.dma_start(wv_t[:], WV)
        wr_t = wpool.tile([128, NJ * D], dt.bfloat16, tag="wr")
        nc.sync.dma_start(wr_t[:], WR)
        cv = wpool.tile([128, 40], dt.float32, tag="cv")
        nc.sync.dma_start(cv[:], CV)
        wo_t = wpool.tile([128, NJ * D], dt.bfloat16, tag="wo")
        nc.sync.dma_start(wo_t[:], WO)

        def states(prefix):
            ts_ = []
            for e in range(NJ):
                t = stp.tile([128, 1], dt.float32, tag=f"{prefix}{e}")
                nc.vector.memset(t[:], 0.0)
                ts_.append(t)
            return ts_

        ekst = states("ekst")   # ek halo carry (scalar engine)
        ast = states("ast")     # a' halo carry (vector)
        alst = states("alst")   # sa scan carry (vector)
        best = states("best")   # sb scan carry (gpsimd)

        def load_x(c):
            t0 = c * TC
            xh = []
            for j in range(NJ):
                x_ = xp.tile([128, TC + 1], dt.bfloat16, tag="xh")
                nc.sync.dma_start(x_[:], XT[j * 128:(j + 1) * 128, t0:t0 + TC + 1])
                xh.append(x_)
            return xh

        def mix_stage(xh):
            """diff + time-mix for k/v/r, all bf16 on DVE."""
            mk_l, mv_l, mr_l = [], [], []
            for j in range(NJ):
                d_ = dfp.tile([128, TC], dt.bfloat16, tag="df")
                nc.vector.tensor_tensor(d_[:], xh[j][:, 1:TC + 1], xh[j][:, 0:TC],
                                        OP.subtract)
                for pi, (pool, lst) in enumerate(((mkp, mk_l), (mvp, mv_l),
                                                  (mrp, mr_l))):
                    m_ = pool.tile([128, TC], dt.bfloat16, tag=f"m{pi}")
                    nc.vector.scalar_tensor_tensor(
                        m_[:], d_[:], cv[:, pi * 8 + j:pi * 8 + j + 1],
                        xh[j][:, 0:TC], OP.mult, OP.add)
                    lst.append(m_)
            return mk_l, mv_l, mr_l

        def chunk_body(c, mixes):
            """k/v/r matmuls + WKV chain for chunk c; returns rw tiles.

            Front half (per e): matmuls, scalar exp's, gpsimd a'.
            Back half (per e, skewed by one): vector scans/num/den/recip,
            gpsimd den2/rw. The skew keeps a'(e) ready before scan_a(e).
            """
            mk_l, mv_l, mr_l = mixes
            rws = []
            front = {}

            def front_half(e):
                acck = kp.tile([128, TC], dt.float32, tag="acck")
                for j in range(NJ):
                    nc.tensor.matmul(
                        acck[:], wk_t[:, j * D + e * 128: j * D + (e + 1) * 128],
                        mk_l[j][:], start=(j == 0), stop=(j == NJ - 1))
                accv = vp.tile([128, TC], dt.float32, tag="accv")
                for j in range(NJ):
                    nc.tensor.matmul(
                        accv[:], wv_t[:, j * D + e * 128: j * D + (e + 1) * 128],
                        mv_l[j][:], start=(j == 0), stop=(j == NJ - 1))
                accr = rp.tile([128, TC], dt.float32, tag="accr")
                for j in range(NJ):
                    nc.tensor.matmul(
                        accr[:], wr_t[:, j * D + e * 128: j * D + (e + 1) * 128],
                        mr_l[j][:], start=(j == 0), stop=(j == NJ - 1))

                # scalar: ek = exp(k) with halo, er = exp(-r), oner = 1+er
                ek = ekp.tile([128, TC + 1], dt.float32, tag="ek")
                nc.scalar.copy(ek[:, 0:1], ekst[e][:])
                nc.scalar.activation(ek[:, 1:TC + 1], acck[:], AF.Exp)
                nc.scalar.copy(ekst[e][:], ek[:, TC:TC + 1])
                er = erp.tile([128, TC], dt.float32, tag="er")
                nc.scalar.activation(er[:], accr[:], AF.Exp, scale=-1.0)
                oner = onep.tile([128, TC], dt.float32, tag="oner")
                nc.scalar.activation(oner[:], er[:], AF.Copy, bias=1.0)
                vsb = vsp.tile([128, TC], dt.float32, tag="vsb")
                nc.scalar.copy(vsb[:], accv[:])

                # gpsimd: a' = ek*v with halo (gpsimd cannot touch PSUM)
                a_ = app.tile([128, TC + 1], dt.float32, tag="a")
                nc.gpsimd.tensor_copy(a_[:, 0:1], ast[e][:])
                nc.gpsimd.tensor_tensor(a_[:, 1:TC + 1], ek[:, 1:TC + 1], vsb[:],
                                        OP.mult)
                nc.gpsimd.tensor_copy(ast[e][:], a_[:, TC:TC + 1])
                front[e] = (ek, a_, oner)

            def back_half(e):
                ek, a_, oner = front.pop(e)
                H = TC // 2
                ewb = cv[:, 24 + e: 25 + e].broadcast_to([128, H])
                # scans split into two chained 256-halves (DVE's >256 path
                # is disproportionately slow: 2030ns vs 2x600ns)
                sa = sap.tile([128, TC], dt.float32, tag="sa")
                nc.vector.tensor_tensor_scan(sa[:, 0:H], ewb, a_[:, 0:H],
                                             alst[e][:], OP.mult, OP.add)
                nc.vector.tensor_tensor_scan(sa[:, H:TC], ewb, a_[:, H:TC],
                                             sa[:, H - 1:H], OP.mult, OP.add)
                nc.vector.tensor_copy(alst[e][:], sa[:, TC - 1:TC])
                sb = sbp.tile([128, TC], dt.float32, tag="sb")
                nc.vector.tensor_tensor_scan(sb[:, 0:H], ewb, ek[:, 0:H],
                                             best[e][:], OP.mult, OP.add)
                nc.vector.tensor_tensor_scan(sb[:, H:TC], ewb, ek[:, H:TC],
                                             sb[:, H - 1:H], OP.mult, OP.add)
                nc.vector.tensor_copy(best[e][:], sb[:, TC - 1:TC])

                eu = cv[:, 32 + e: 33 + e]
                num = nump.tile([128, TC], dt.float32, tag="num")
                nc.vector.scalar_tensor_tensor(num[:], a_[:, 1:TC + 1], eu, sa[:],
                                               OP.mult, OP.add)
                den = denp.tile([128, TC], dt.float32, tag="den")
                nc.vector.scalar_tensor_tensor(den[:], ek[:, 1:TC + 1], eu, sb[:],
                                               OP.mult, OP.add)
                # gpsimd: den2 = den*(1+er); vector: recip; gpsimd: rw
                dn2 = dn2p.tile([128, TC], dt.float32, tag="dn2")
                nc.gpsimd.tensor_tensor(dn2[:], den[:], oner[:], OP.mult)
                nc.vector.reciprocal_approx_fast(dn2[:], dn2[:])
                rw = rwp.tile([128, TC], dt.bfloat16, tag="rw")
                nc.gpsimd.tensor_tensor(rw[:], num[:], dn2[:], OP.mult)
                rws.append(rw)

            for e in range(NJ):
                front_half(e)
                if e > 0:
                    back_half(e - 1)
            back_half(NJ - 1)
            return rws

        def out_stage(c, rws):
            """out = rw^T @ Wo^T for chunk c, straight to DRAM."""
            t0 = c * TC
            for ts_ in range(NTS):
                for eh in range(2):
                    op = outp.tile([128, 512], dt.float32, tag="op")
                    for j in range(NJ):
                        nc.tensor.matmul(
                            op[:], rws[j][:, ts_ * 128:(ts_ + 1) * 128],
                            wo_t[:, j * D + eh * 512: j * D + (eh + 1) * 512],
                            start=(j == 0), stop=(j == NJ - 1))
                    oc = ocp.tile([128, 512], dt.float32, tag="oc")
                    nc.scalar.copy(oc[:], op[:])
                    nc.scalar.dma_start(
                        O[t0 + ts_ * 128: t0 + (ts_ + 1) * 128,
                          eh * 512:(eh + 1) * 512], oc[:])

        # ---- pipelined chunk loop ----
        xh0 = load_x(0)
        xh1 = load_x(1)
        mixes = mix_stage(xh0)
        xh_next = xh1
        rws_prev = None
        for c in range(nch):
            if c + 2 < nch:
                xh_fut = load_x(c + 2)
            else:
                xh_fut = None
            if c + 1 < nch:
                mixes_next = mix_stage(xh_next)
            rws = chunk_body(c, mixes)
            if rws_prev is not None:
                out_stage(c - 1, rws_prev)
            rws_prev = rws
            if c + 1 < nch:
                mixes = mixes_next
                xh_next = xh_fut
        out_stage(nch - 1, rws_prev)


def pack_inputs(x_slice, time_decay, time_first, time_mix_k, time_mix_v,
                time_mix_r, Wk, Wv, Wr, Wo):
    """Host-side packing for one core. x_slice: [T, D] fp32."""
    import ml_dtypes
    bf16 = ml_dtypes.bfloat16

    def packw(W):
        return np.ascontiguousarray(
            W.T.reshape(NJ, 128, D).transpose(1, 0, 2).reshape(128, NJ * D)
        ).astype(bf16)

    def packv(v):
        return np.ascontiguousarray(v.reshape(NJ, 128).T).astype(np.float32)

    T = x_slice.shape[0]
    xt = np.zeros((D, T + 1), dtype=bf16)
    xt[:, 1:] = x_slice.T.astype(bf16)

    mk = time_mix_k.reshape(D).astype(np.float32)
    mv = time_mix_v.reshape(D).astype(np.float32)
    mr = time_mix_r.reshape(D).astype(np.float32)
    ew = np.exp(-np.exp(time_decay.astype(np.float32))).astype(np.float32)
    eu = np.exp(time_first.astype(np.float32).reshape(D)).astype(np.float32)
    cv = np.concatenate([
        packv(mk), packv(mv), packv(mr), packv(ew), packv(eu)],
        axis=1).astype(np.float32)
    return {
        "xt": np.ascontiguousarray(xt),
        "wk": packw(Wk), "wv": packw(Wv), "wr": packw(Wr), "wo": packw(Wo),
        "cv": cv,
    }


# ---------------------------------------------------------------------------
# Harness entry point: full inputs in, full output out, 8-way batch-parallel.
# ---------------------------------------------------------------------------
_CACHE = {}
_last_exec_time_ns = None


def _get_program(n_cores):
    key = ("prog", n_cores)
    if key not in _CACHE:
        nc = bacc.Bacc("TRN2", target_bir_lowering=False, debug=False,
                       num_devices=n_cores)
        build(nc, T=4096)
        nc.compile()
        _CACHE[key] = nc
    return _CACHE[key]


def kernel(x, time_decay, time_first, time_mix_k, time_mix_v, time_mix_r,
           Wk, Wv, Wr, Wo):
    """WKV attention: x [8, 4096, 1024] fp32 -> out [8, 4096, 1024] fp32.

    Shards batch across the 8 NeuronCores (one batch element per core).
    """
    global _last_exec_time_ns
    import os
    from concourse import bass_utils

    x = np.asarray(x, dtype=np.float32)
    B = x.shape[0]
    base = pack_inputs(x[0], np.asarray(time_decay), np.asarray(time_first),
                       np.asarray(time_mix_k), np.asarray(time_mix_v),
                       np.asarray(time_mix_r), np.asarray(Wk), np.asarray(Wv),
                       np.asarray(Wr), np.asarray(Wo))
    import ml_dtypes
    bf16 = ml_dtypes.bfloat16
    in_maps = []
    for b in range(B):
        m = dict(base)
        xt = np.zeros((D, x.shape[1] + 1), dtype=bf16)
        xt[:, 1:] = x[b].T.astype(bf16)
        m["xt"] = np.ascontiguousarray(xt)
        in_maps.append(m)

    nc = _get_program(B)
    trace = os.environ.get("WKV_TRACE", "0") == "1"
    r = bass_utils.run_bass_kernel_spmd(nc, in_maps, core_ids=list(range(B)),
                                        trace=trace)
    _last_exec_time_ns = r.exec_time_ns
    return np.stack([r.results[b]["o"] for b in range(B)]).astype(np.float32)


# revision 19
# speedup vs baseline: 1.2462x; 1.2004x over previous
"""RWKV WKV attention kernel for TRN2 (Bass/Tile), batch-parallel over 8 cores.

v3: host-transposed bf16 x (leading zero halo column), bf16 matmuls,
fused output projection (no DRAM round-trip), scans split across
Vector and GpSimd, sigmoid via the Exp table, den via one stt.

Per core (one batch element):
  chunk loop over T in TC=512 steps:
    DMA xT halo tile [128, TC+1] per d-group j (halo = col t0-1).
    diff/mix (bf16, DVE) -> k,v,r projections (bf16 matmul, fp32 PSUM).
    ek = exp(k); a' = ek*v; scans sa (DVE) / sb (GpSimd);
    num = a'*e^u + sa; den = ek*e^u + sb; den2 = (1+e^-r)*den;
    rw = num * recip(den2) (bf16) -> out = rw^T @ Wo^T (fused, per chunk).

Host-packed weights [128, 8*1024] bf16: arr[p, j*1024+e] = W[e, j*128+p].
cv [128, 40] fp32 (col j of group g = channels j*128..j*128+127):
  0-7 mk, 8-15 mv, 16-23 mr, 24-31 ew=exp(-exp(time_decay)), 32-39 eu=e^u.
"""
import sys
for p in ("/opt/trn_rl_repo",):
    if p not in sys.path:
        sys.path.insert(0, p)

import numpy as np
from contextlib import ExitStack

import concourse.bass as bass
import concourse.tile as tile
from concourse import bacc, mybir

dt = mybir.dt
AF = mybir.ActivationFunctionType
OP = mybir.AluOpType

D = 1024
NJ = D // 128  # 8 channel chunks


def build(nc, T=4096, TC=512):
    nch = T // TC
    NTS = TC // 128

    XT = nc.dram_tensor("xt", [D, T + 1], dt.bfloat16, kind="ExternalInput").ap()
    WK = nc.dram_tensor("wk", [128, NJ * D], dt.bfloat16, kind="ExternalInput").ap()
    WV = nc.dram_tensor("wv", [128, NJ * D], dt.bfloat16, kind="ExternalInput").ap()
    WR = nc.dram_tensor("wr", [128, NJ * D], dt.bfloat16, kind="ExternalInput").ap()
    WO = nc.dram_tensor("wo", [128, NJ * D], dt.bfloat16, kind="ExternalInput").ap()
    CV = nc.dram_tensor("cv", [128, 64], dt.float32, kind="ExternalInput").ap()
    O = nc.dram_tensor("o", [T, D], dt.float32, kind="ExternalOutput").ap()

    with tile.TileContext(nc) as tc, ExitStack() as ctx:
        wpool = ctx.enter_context(tc.tile_pool(name="wpool", bufs=1))
        xp = ctx.enter_context(tc.tile_pool(name="xp", bufs=2 * NJ + 2))
        yp = ctx.enter_context(tc.tile_pool(name="yp", bufs=3))
        mkp = ctx.enter_context(tc.tile_pool(name="mkp", bufs=2 * NJ))
        mvp = ctx.enter_context(tc.tile_pool(name="mvp", bufs=2 * NJ))
        mrp = ctx.enter_context(tc.tile_pool(name="mrp", bufs=2 * NJ))
        kp = ctx.enter_context(tc.tile_pool(name="kp", bufs=2, space="PSUM"))
        vp = ctx.enter_context(tc.tile_pool(name="vp", bufs=2, space="PSUM"))
        rp = ctx.enter_context(tc.tile_pool(name="rp", bufs=2, space="PSUM"))
        outp = ctx.enter_context(tc.tile_pool(name="outp", bufs=2, space="PSUM"))
        ekp = ctx.enter_context(tc.tile_pool(name="ekp", bufs=3))
        app = ctx.enter_context(tc.tile_pool(name="app", bufs=3))
        onep = ctx.enter_context(tc.tile_pool(name="onep", bufs=3))
        vsp = ctx.enter_context(tc.tile_pool(name="vsp", bufs=2))
        sap = ctx.enter_context(tc.tile_pool(name="sap", bufs=2))
        sbp = ctx.enter_context(tc.tile_pool(name="sbp", bufs=2))
        nump = ctx.enter_context(tc.tile_pool(name="nump", bufs=2))
        denp = ctx.enter_context(tc.tile_pool(name="denp", bufs=2))
        dn2p = ctx.enter_context(tc.tile_pool(name="dn2p", bufs=2))
        erp = ctx.enter_context(tc.tile_pool(name="erp", bufs=2))
        rwp = ctx.enter_context(tc.tile_pool(name="rwp", bufs=2 * NJ))
        ocp = ctx.enter_context(tc.tile_pool(name="ocp", bufs=2))
        stp = ctx.enter_context(tc.tile_pool(name="stp", bufs=1))

        wk_t = wpool.tile([128, NJ * D], dt.bfloat16, tag="wk")
        nc.sync.dma_start(wk_t[:], WK)
        wv_t = wpool.tile([128, NJ * D], dt.bfloat16, tag="wv")
        nc.sync.dma_start(wv_t[:], WV)
        wr_t = wpool.tile([128, NJ * D], dt.bfloat16, tag="wr")
        nc.sync.dma_start(wr_t[:], WR)
        cv = wpool.tile([128, 64], dt.float32, tag="cv")
        nc.sync.dma_start(cv[:], CV)
        wo_t = wpool.tile([128, NJ * D], dt.bfloat16, tag="wo")
        nc.sync.dma_start(wo_t[:], WO)

        def states(prefix):
            ts_ = []
            for e in range(NJ):
                t = stp.tile([128, 1], dt.float32, tag=f"{prefix}{e}")
                nc.vector.memset(t[:], 0.0)
                ts_.append(t)
            return ts_

        ekst = states("ekst")   # ek halo carry (scalar engine)
        ast = states("ast")     # a' halo carry (vector)
        alst = states("alst")   # sa scan carry (vector)
        best = states("best")   # sb scan carry (gpsimd)

        def load_x(c):
            t0 = c * TC
            xh = []
            for j in range(NJ):
                x_ = xp.tile([128, TC + 1], dt.bfloat16, tag="xh")
                nc.sync.dma_start(x_[:], XT[j * 128:(j + 1) * 128, t0:t0 + TC + 1])
                xh.append(x_)
            return xh

        def mix_stage(xh):
            """time-mix for k/v/r: y=(1-m)*x_prev on scalar, stt on DVE."""
            mk_l, mv_l, mr_l = [], [], []
            for j in range(NJ):
                for pi, (pool, lst) in enumerate(((mkp, mk_l), (mvp, mv_l),
                                                  (mrp, mr_l))):
                    y_ = yp.tile([128, TC], dt.bfloat16, tag=f"y{pi}")
                    nc.scalar.activation(
                        y_[:], xh[j][:, 0:TC], AF.Copy,
                        scale=cv[:, 40 + pi * 8 + j: 41 + pi * 8 + j])
                    m_ = pool.tile([128, TC], dt.bfloat16, tag=f"m{pi}")
                    nc.vector.scalar_tensor_tensor(
                        m_[:], xh[j][:, 1:TC + 1], cv[:, pi * 8 + j:pi * 8 + j + 1],
                        y_[:], OP.mult, OP.add)
                    lst.append(m_)
            return mk_l, mv_l, mr_l

        def chunk_body(c, mixes, mix_next, mixes_out):
            """k/v/r matmuls + WKV chain for chunk c; returns rw tiles.

            Front half (per e): matmuls, scalar exp's, gpsimd a'.
            Back half (per e, skewed by one): vector scans/num/den/recip,
            gpsimd den2/rw. The skew keeps a'(e) ready before scan_a(e).
            mix_next: xh tiles for chunk c+1; its mix ops are issued after
            front_half(0) so they don't delay this chunk's chain.
            """
            mk_l, mv_l, mr_l = mixes
            rws = []
            front = {}

            def front_half(e):
                acck = kp.tile([128, TC], dt.float32, tag="acck")
                for j in range(NJ):
                    nc.tensor.matmul(
                        acck[:], wk_t[:, j * D + e * 128: j * D + (e + 1) * 128],
                        mk_l[j][:], start=(j == 0), stop=(j == NJ - 1))
                accv = vp.tile([128, TC], dt.float32, tag="accv")
                for j in range(NJ):
                    nc.tensor.matmul(
                        accv[:], wv_t[:, j * D + e * 128: j * D + (e + 1) * 128],
                        mv_l[j][:], start=(j == 0), stop=(j == NJ - 1))
                accr = rp.tile([128, TC], dt.float32, tag="accr")
                for j in range(NJ):
                    nc.tensor.matmul(
                        accr[:], wr_t[:, j * D + e * 128: j * D + (e + 1) * 128],
                        mr_l[j][:], start=(j == 0), stop=(j == NJ - 1))

                # scalar: ek = exp(k) with halo, er = exp(-r), oner = 1+er
                ek = ekp.tile([128, TC + 1], dt.float32, tag="ek")
                nc.scalar.copy(ek[:, 0:1], ekst[e][:])
                nc.scalar.activation(ek[:, 1:TC + 1], acck[:], AF.Exp)
                nc.scalar.copy(ekst[e][:], ek[:, TC:TC + 1])
                er = erp.tile([128, TC], dt.float32, tag="er")
                nc.scalar.activation(er[:], accr[:], AF.Exp, scale=-1.0)
                oner = onep.tile([128, TC], dt.float32, tag="oner")
                nc.scalar.activation(oner[:], er[:], AF.Copy, bias=1.0)
                vsb = vsp.tile([128, TC], dt.float32, tag="vsb")
                nc.scalar.copy(vsb[:], accv[:])

                # gpsimd: a' = ek*v with halo (gpsimd cannot touch PSUM)
                a_ = app.tile([128, TC + 1], dt.float32, tag="a")
                nc.gpsimd.tensor_copy(a_[:, 0:1], ast[e][:])
                nc.gpsimd.tensor_tensor(a_[:, 1:TC + 1], ek[:, 1:TC + 1], vsb[:],
                                        OP.mult)
                nc.gpsimd.tensor_copy(ast[e][:], a_[:, TC:TC + 1])
                front[e] = (ek, a_, oner)

            def back_half(e):
                ek, a_, oner = front.pop(e)
                ewb = cv[:, 24 + e: 25 + e].broadcast_to([128, TC])
                sa = sap.tile([128, TC], dt.float32, tag="sa")
                nc.vector.tensor_tensor_scan(sa[:], ewb, a_[:, 0:TC], alst[e][:],
                                             OP.mult, OP.add)
                nc.vector.tensor_copy(alst[e][:], sa[:, TC - 1:TC])
                sb = sbp.tile([128, TC], dt.float32, tag="sb")
                nc.vector.tensor_tensor_scan(sb[:], ewb, ek[:, 0:TC], best[e][:],
                                             OP.mult, OP.add)
                nc.vector.tensor_copy(best[e][:], sb[:, TC - 1:TC])

                eu = cv[:, 32 + e: 33 + e]
                num = nump.tile([128, TC], dt.float32, tag="num")
                nc.vector.scalar_tensor_tensor(num[:], a_[:, 1:TC + 1], eu, sa[:],
                                               OP.mult, OP.add)
                den = denp.tile([128, TC], dt.float32, tag="den")
                nc.vector.scalar_tensor_tensor(den[:], ek[:, 1:TC + 1], eu, sb[:],
                                               OP.mult, OP.add)
                # gpsimd: den2 = den*(1+er); vector: recip; gpsimd: rw
                dn2 = dn2p.tile([128, TC], dt.float32, tag="dn2")
                nc.gpsimd.tensor_tensor(dn2[:], den[:], oner[:], OP.mult)
                nc.vector.reciprocal_approx_fast(dn2[:], dn2[:])
                rw = rwp.tile([128, TC], dt.bfloat16, tag="rw")
                nc.gpsimd.tensor_tensor(rw[:], num[:], dn2[:], OP.mult)
                rws.append(rw)

            for e in range(NJ):
                front_half(e)
                if e == 0 and mix_next is not None:
                    mixes_out.append(mix_stage(mix_next))
                if e > 0:
                    back_half(e - 1)
            back_half(NJ - 1)
            return rws

        def out_stage(c, rws):
            """out = rw^T @ Wo^T for chunk c, straight to DRAM."""
            t0 = c * TC
            for ts_ in range(NTS):
                for eh in range(2):
                    op = outp.tile([128, 512], dt.float32, tag="op")
                    for j in range(NJ):
                        nc.tensor.matmul(
                            op[:], rws[j][:, ts_ * 128:(ts_ + 1) * 128],
                            wo_t[:, j * D + eh * 512: j * D + (eh + 1) * 512],
                            start=(j == 0), stop=(j == NJ - 1))
                    oc = ocp.tile([128, 512], dt.float32, tag="oc")
                    nc.scalar.copy(oc[:], op[:])
                    nc.scalar.dma_start(
                        O[t0 + ts_ * 128: t0 + (ts_ + 1) * 128,
                          eh * 512:(eh + 1) * 512], oc[:])

        # ---- pipelined chunk loop ----
        xh0 = load_x(0)
        xh1 = load_x(1)
        mixes = mix_stage(xh0)
        xh_next = xh1
        rws_prev = None
        for c in range(nch):
            if c + 2 < nch:
                xh_fut = load_x(c + 2)
            else:
                xh_fut = None
            mixes_out = []
            rws = chunk_body(c, mixes,
                             xh_next if c + 1 < nch else None, mixes_out)
            if rws_prev is not None:
                out_stage(c - 1, rws_prev)
            rws_prev = rws
            if c + 1 < nch:
                mixes = mixes_out[0]
                xh_next = xh_fut
        out_stage(nch - 1, rws_prev)


def pack_inputs(x_slice, time_decay, time_first, time_mix_k, time_mix_v,
                time_mix_r, Wk, Wv, Wr, Wo):
    """Host-side packing for one core. x_slice: [T, D] fp32."""
    import ml_dtypes
    bf16 = ml_dtypes.bfloat16

    def packw(W):
        return np.ascontiguousarray(
            W.T.reshape(NJ, 128, D).transpose(1, 0, 2).reshape(128, NJ * D)
        ).astype(bf16)

    def packv(v):
        return np.ascontiguousarray(v.reshape(NJ, 128).T).astype(np.float32)

    T = x_slice.shape[0]
    xt = np.zeros((D, T + 1), dtype=bf16)
    xt[:, 1:] = x_slice.T.astype(bf16)

    mk = time_mix_k.reshape(D).astype(np.float32)
    mv = time_mix_v.reshape(D).astype(np.float32)
    mr = time_mix_r.reshape(D).astype(np.float32)
    ew = np.exp(-np.exp(time_decay.astype(np.float32))).astype(np.float32)
    eu = np.exp(time_first.astype(np.float32).reshape(D)).astype(np.float32)
    cv = np.concatenate([
        packv(mk), packv(mv), packv(mr), packv(ew), packv(eu),
        packv(1.0 - mk), packv(1.0 - mv), packv(1.0 - mr)],
        axis=1).astype(np.float32)
    return {
        "xt": np.ascontiguousarray(xt),
        "wk": packw(Wk), "wv": packw(Wv), "wr": packw(Wr), "wo": packw(Wo),
        "cv": cv,
    }


# ---------------------------------------------------------------------------
# Harness entry point: full inputs in, full output out, 8-way batch-parallel.
# ---------------------------------------------------------------------------
_CACHE = {}
_last_exec_time_ns = None


def _get_program(n_cores):
    key = ("prog", n_cores)
    if key not in _CACHE:
        nc = bacc.Bacc("TRN2", target_bir_lowering=False, debug=False,
                       num_devices=n_cores)
        build(nc, T=4096)
        nc.compile()
        _CACHE[key] = nc
    return _CACHE[key]


def kernel(x, time_decay, time_first, time_mix_k, time_mix_v, time_mix_r,
           Wk, Wv, Wr, Wo):
    """WKV attention: x [8, 4096, 1024] fp32 -> out [8, 4096, 1024] fp32.

    Shards batch across the 8 NeuronCores (one batch element per core).
    """
    global _last_exec_time_ns
    import os
    from concourse import bass_utils

    x = np.asarray(x, dtype=np.float32)
    B = x.shape[0]
    base = pack_inputs(x[0], np.asarray(time_decay), np.asarray(time_first),
                       np.asarray(time_mix_k), np.asarray(time_mix_v),
                       np.asarray(time_mix_r), np.asarray(Wk), np.asarray(Wv),
                       np.asarray(Wr), np.asarray(Wo))
    import ml_dtypes
    bf16 = ml_dtypes.bfloat16
    in_maps = []
    for b in range(B):
        m = dict(base)
        xt = np.zeros((D, x.shape[1] + 1), dtype=bf16)
        xt[:, 1:] = x[b].T.astype(bf16)
        m["xt"] = np.ascontiguousarray(xt)
        in_maps.append(m)

    nc = _get_program(B)
    trace = os.environ.get("WKV_TRACE", "0") == "1"
    r = bass_utils.run_bass_kernel_spmd(nc, in_maps, core_ids=list(range(B)),
                                        trace=trace)
    _last_exec_time_ns = r.exec_time_ns
    return np.stack([r.results[b]["o"] for b in range(B)]).astype(np.float32)


# revision 23
# speedup vs baseline: 1.2608x; 1.0117x over previous
"""RWKV WKV attention kernel for TRN2 (Bass/Tile), batch-parallel over 8 cores.

v3: host-transposed bf16 x (leading zero halo column), bf16 matmuls,
fused output projection (no DRAM round-trip), scans split across
Vector and GpSimd, sigmoid via the Exp table, den via one stt.

Per core (one batch element):
  chunk loop over T in TC=512 steps:
    DMA xT halo tile [128, TC+1] per d-group j (halo = col t0-1).
    diff/mix (bf16, DVE) -> k,v,r projections (bf16 matmul, fp32 PSUM).
    ek = exp(k); a' = ek*v; scans sa (DVE) / sb (GpSimd);
    num = a'*e^u + sa; den = ek*e^u + sb; den2 = (1+e^-r)*den;
    rw = num * recip(den2) (bf16) -> out = rw^T @ Wo^T (fused, per chunk).

Host-packed weights [128, 8*1024] bf16: arr[p, j*1024+e] = W[e, j*128+p].
cv [128, 40] fp32 (col j of group g = channels j*128..j*128+127):
  0-7 mk, 8-15 mv, 16-23 mr, 24-31 ew=exp(-exp(time_decay)), 32-39 eu=e^u.
"""
import sys
for p in ("/opt/trn_rl_repo",):
    if p not in sys.path:
        sys.path.insert(0, p)

import numpy as np
from contextlib import ExitStack

import concourse.bass as bass
import concourse.tile as tile
from concourse import bacc, mybir

dt = mybir.dt
AF = mybir.ActivationFunctionType
OP = mybir.AluOpType

D = 1024
NJ = D // 128  # 8 channel chunks


def build(nc, T=4096, TC=512):
    nch = T // TC
    NTS = TC // 128

    XT = nc.dram_tensor("xt", [D, T + 1], dt.bfloat16, kind="ExternalInput").ap()
    WK = nc.dram_tensor("wk", [128, NJ * D], dt.bfloat16, kind="ExternalInput").ap()
    WV = nc.dram_tensor("wv", [128, NJ * D], dt.bfloat16, kind="ExternalInput").ap()
    WR = nc.dram_tensor("wr", [128, NJ * D], dt.bfloat16, kind="ExternalInput").ap()
    WO = nc.dram_tensor("wo", [128, NJ * D], dt.bfloat16, kind="ExternalInput").ap()
    CV = nc.dram_tensor("cv", [128, 64], dt.float32, kind="ExternalInput").ap()
    O = nc.dram_tensor("o", [T, D], dt.float32, kind="ExternalOutput").ap()

    with tile.TileContext(nc) as tc, ExitStack() as ctx:
        wpool = ctx.enter_context(tc.tile_pool(name="wpool", bufs=1))
        xp = ctx.enter_context(tc.tile_pool(name="xp", bufs=2 * NJ + 2))
        yp = ctx.enter_context(tc.tile_pool(name="yp", bufs=3))
        mkp = ctx.enter_context(tc.tile_pool(name="mkp", bufs=2 * NJ))
        mvp = ctx.enter_context(tc.tile_pool(name="mvp", bufs=2 * NJ))
        mrp = ctx.enter_context(tc.tile_pool(name="mrp", bufs=2 * NJ))
        kp = ctx.enter_context(tc.tile_pool(name="kp", bufs=1, space="PSUM"))
        vp = ctx.enter_context(tc.tile_pool(name="vp", bufs=2, space="PSUM"))
        rp = ctx.enter_context(tc.tile_pool(name="rp", bufs=2, space="PSUM"))
        outp = ctx.enter_context(tc.tile_pool(name="outp", bufs=3, space="PSUM"))
        ekp = ctx.enter_context(tc.tile_pool(name="ekp", bufs=3))
        app = ctx.enter_context(tc.tile_pool(name="app", bufs=3))
        onep = ctx.enter_context(tc.tile_pool(name="onep", bufs=3))
        vsp = ctx.enter_context(tc.tile_pool(name="vsp", bufs=2))
        sap = ctx.enter_context(tc.tile_pool(name="sap", bufs=2))
        sbp = ctx.enter_context(tc.tile_pool(name="sbp", bufs=2))
        nump = ctx.enter_context(tc.tile_pool(name="nump", bufs=2))
        denp = ctx.enter_context(tc.tile_pool(name="denp", bufs=2))
        dn2p = ctx.enter_context(tc.tile_pool(name="dn2p", bufs=2))
        erp = ctx.enter_context(tc.tile_pool(name="erp", bufs=2))
        rwp = ctx.enter_context(tc.tile_pool(name="rwp", bufs=2 * NJ))
        ocp = ctx.enter_context(tc.tile_pool(name="ocp", bufs=2))
        stp = ctx.enter_context(tc.tile_pool(name="stp", bufs=1))

        def load_w(src, tag):
            t = wpool.tile([128, NJ * D], dt.bfloat16, tag=tag, name=tag)
            for q in range(4):
                s = q * (NJ * D // 4)
                nc.sync.dma_start(t[:, s:s + NJ * D // 4],
                                  src[:, s:s + NJ * D // 4])
            return t

        wk_t = load_w(WK, "wk")
        cv = wpool.tile([128, 64], dt.float32, tag="cv")
        nc.sync.dma_start(cv[:], CV)
        wv_t = load_w(WV, "wv")
        wr_t = load_w(WR, "wr")
        wo_t = load_w(WO, "wo")

        def states(prefix):
            ts_ = []
            for e in range(NJ):
                t = stp.tile([128, 1], dt.float32, tag=f"{prefix}{e}")
                nc.vector.memset(t[:], 0.0)
                ts_.append(t)
            return ts_

        ekst = states("ekst")   # ek halo carry (scalar engine)
        ast = states("ast")     # a' halo carry (vector)
        alst = states("alst")   # sa scan carry (vector)
        best = states("best")   # sb scan carry (gpsimd)

        def load_x(c):
            t0 = c * TC
            xh = []
            for j in range(NJ):
                x_ = xp.tile([128, TC + 1], dt.bfloat16, tag="xh")
                nc.sync.dma_start(x_[:], XT[j * 128:(j + 1) * 128, t0:t0 + TC + 1])
                xh.append(x_)
            return xh

        def mix_one(xh, j, out_lists):
            """time-mix for k/v/r of one d-group: y=(1-m)*x_prev on scalar,
            stt on DVE. Issued per-e inside chunk_body so the y ops don't
            block the scalar queue ahead of the exp's."""
            mk_l, mv_l, mr_l = out_lists
            for pi, lst, pool in ((0, mk_l, mkp), (1, mv_l, mvp), (2, mr_l, mrp)):
                y_ = yp.tile([128, TC], dt.bfloat16, tag=f"y{pi}")
                nc.scalar.activation(
                    y_[:], xh[j][:, 0:TC], AF.Copy,
                    scale=cv[:, 40 + pi * 8 + j: 41 + pi * 8 + j])
                m_ = pool.tile([128, TC], dt.bfloat16, tag=f"m{pi}")
                nc.vector.scalar_tensor_tensor(
                    m_[:], xh[j][:, 1:TC + 1], cv[:, pi * 8 + j:pi * 8 + j + 1],
                    y_[:], OP.mult, OP.add)
                lst.append(m_)

        def mix_stage(xh):
            out = ([], [], [])
            for j in range(NJ):
                mix_one(xh, j, out)
            return out

        def chunk_body(c, mixes, mix_next, mixes_out):
            """k/v/r matmuls + WKV chain for chunk c; returns rw tiles.

            Front half (per e): matmuls, scalar exp's, gpsimd a'.
            Back half (per e, skewed by one): vector scans/num/den/recip,
            gpsimd den2/rw. The skew keeps a'(e) ready before scan_a(e).
            mix_next: xh tiles for chunk c+1; its mix ops are issued after
            front_half(0) so they don't delay this chunk's chain.
            """
            mk_l, mv_l, mr_l = mixes
            rws = []
            front = {}

            def front_half(e):
                acck = kp.tile([128, TC], dt.float32, tag="acck")
                for j in range(NJ):
                    nc.tensor.matmul(
                        acck[:], wk_t[:, j * D + e * 128: j * D + (e + 1) * 128],
                        mk_l[j][:], start=(j == 0), stop=(j == NJ - 1))
                accv = vp.tile([128, TC], dt.float32, tag="accv")
                for j in range(NJ):
                    nc.tensor.matmul(
                        accv[:], wv_t[:, j * D + e * 128: j * D + (e + 1) * 128],
                        mv_l[j][:], start=(j == 0), stop=(j == NJ - 1))
                accr = rp.tile([128, TC], dt.float32, tag="accr")
                for j in range(NJ):
                    nc.tensor.matmul(
                        accr[:], wr_t[:, j * D + e * 128: j * D + (e + 1) * 128],
                        mr_l[j][:], start=(j == 0), stop=(j == NJ - 1))

                # scalar: ek = exp(k) with halo, er = exp(-r), oner = 1+er
                ek = ekp.tile([128, TC + 1], dt.float32, tag="ek")
                nc.scalar.copy(ek[:, 0:1], ekst[e][:])
                nc.scalar.activation(ek[:, 1:TC + 1], acck[:], AF.Exp)
                nc.scalar.copy(ekst[e][:], ek[:, TC:TC + 1])
                er = erp.tile([128, TC], dt.float32, tag="er")
                nc.scalar.activation(er[:], accr[:], AF.Exp, scale=-1.0)
                oner = onep.tile([128, TC], dt.float32, tag="oner")
                nc.scalar.activation(oner[:], er[:], AF.Copy, bias=1.0)
                vsb = vsp.tile([128, TC], dt.float32, tag="vsb")
                nc.scalar.copy(vsb[:], accv[:])

                # gpsimd: a' = ek*v with halo (gpsimd cannot touch PSUM)
                a_ = app.tile([128, TC + 1], dt.float32, tag="a")
                nc.gpsimd.tensor_copy(a_[:, 0:1], ast[e][:])
                nc.gpsimd.tensor_tensor(a_[:, 1:TC + 1], ek[:, 1:TC + 1], vsb[:],
                                        OP.mult)
                nc.gpsimd.tensor_copy(ast[e][:], a_[:, TC:TC + 1])
                front[e] = (ek, a_, oner)

            def back_half(e):
                ek, a_, oner = front.pop(e)
                ewb = cv[:, 24 + e: 25 + e].broadcast_to([128, TC])
                sa = sap.tile([128, TC], dt.float32, tag="sa")
                nc.vector.tensor_tensor_scan(sa[:], ewb, a_[:, 0:TC], alst[e][:],
                                             OP.mult, OP.add)
                nc.vector.tensor_copy(alst[e][:], sa[:, TC - 1:TC])
                sb = sbp.tile([128, TC], dt.float32, tag="sb")
                nc.vector.tensor_tensor_scan(sb[:], ewb, ek[:, 0:TC], best[e][:],
                                             OP.mult, OP.add)
                nc.vector.tensor_copy(best[e][:], sb[:, TC - 1:TC])

                eu = cv[:, 32 + e: 33 + e]
                num = nump.tile([128, TC], dt.float32, tag="num")
                nc.vector.scalar_tensor_tensor(num[:], a_[:, 1:TC + 1], eu, sa[:],
                                               OP.mult, OP.add)
                den = denp.tile([128, TC], dt.float32, tag="den")
                nc.vector.scalar_tensor_tensor(den[:], ek[:, 1:TC + 1], eu, sb[:],
                                               OP.mult, OP.add)
                # gpsimd: den2 = den*(1+er); vector: recip; gpsimd: rw
                dn2 = dn2p.tile([128, TC], dt.float32, tag="dn2")
                nc.gpsimd.tensor_tensor(dn2[:], den[:], oner[:], OP.mult)
                nc.vector.reciprocal_approx_fast(dn2[:], dn2[:])
                rw = rwp.tile([128, TC], dt.bfloat16, tag="rw")
                nc.gpsimd.tensor_tensor(rw[:], num[:], dn2[:], OP.mult)
                rws.append(rw)

            if mix_next is not None:
                mixes_out.append(([], [], []))
            for e in range(NJ):
                front_half(e)
                if mix_next is not None:
                    mix_one(mix_next, e, mixes_out[0])
                if e > 0:
                    back_half(e - 1)
            back_half(NJ - 1)
            return rws

        def out_stage(c, rws):
            """out = rw^T @ Wo^T for chunk c, straight to DRAM."""
            t0 = c * TC
            for ts_ in range(NTS):
                for eh in range(2):
                    op = outp.tile([128, 512], dt.float32, tag="op")
                    for j in range(NJ):
                        nc.tensor.matmul(
                            op[:], rws[j][:, ts_ * 128:(ts_ + 1) * 128],
                            wo_t[:, j * D + eh * 512: j * D + (eh + 1) * 512],
                            start=(j == 0), stop=(j == NJ - 1))
                    oc = ocp.tile([128, 512], dt.float32, tag="oc")
                    nc.scalar.copy(oc[:], op[:])
                    nc.scalar.dma_start(
                        O[t0 + ts_ * 128: t0 + (ts_ + 1) * 128,
                          eh * 512:(eh + 1) * 512], oc[:])

        # ---- pipelined chunk loop ----
        xh0 = load_x(0)
        xh1 = load_x(1)
        mixes = mix_stage(xh0)
        xh_next = xh1
        rws_prev = None
        for c in range(nch):
            if c + 2 < nch:
                xh_fut = load_x(c + 2)
            else:
                xh_fut = None
            mixes_out = []
            rws = chunk_body(c, mixes,
                             xh_next if c + 1 < nch else None, mixes_out)
            if rws_prev is not None:
                out_stage(c - 1, rws_prev)
            rws_prev = rws
            if c + 1 < nch:
                mixes = mixes_out[0]
                xh_next = xh_fut
        out_stage(nch - 1, rws_prev)


def pack_inputs(x_slice, time_decay, time_first, time_mix_k, time_mix_v,
                time_mix_r, Wk, Wv, Wr, Wo):
    """Host-side packing for one core. x_slice: [T, D] fp32."""
    import ml_dtypes
    bf16 = ml_dtypes.bfloat16

    def packw(W):
        return np.ascontiguousarray(
            W.T.reshape(NJ, 128, D).transpose(1, 0, 2).reshape(128, NJ * D)
        ).astype(bf16)

    def packv(v):
        return np.ascontiguousarray(v.reshape(NJ, 128).T).astype(np.float32)

    T = x_slice.shape[0]
    xt = np.zeros((D, T + 1), dtype=bf16)
    xt[:, 1:] = x_slice.T.astype(bf16)

    mk = time_mix_k.reshape(D).astype(np.float32)
    mv = time_mix_v.reshape(D).astype(np.float32)
    mr = time_mix_r.reshape(D).astype(np.float32)
    ew = np.exp(-np.exp(time_decay.astype(np.float32))).astype(np.float32)
    eu = np.exp(time_first.astype(np.float32).reshape(D)).astype(np.float32)
    cv = np.concatenate([
        packv(mk), packv(mv), packv(mr), packv(ew), packv(eu),
        packv(1.0 - mk), packv(1.0 - mv), packv(1.0 - mr)],
        axis=1).astype(np.float32)
    return {
        "xt": np.ascontiguousarray(xt),
        "wk": packw(Wk), "wv": packw(Wv), "wr": packw(Wr), "wo": packw(Wo),
        "cv": cv,
    }


# ---------------------------------------------------------------------------
# Harness entry point: full inputs in, full output out, 8-way batch-parallel.
# ---------------------------------------------------------------------------
_CACHE = {}
_last_exec_time_ns = None


def _get_program(n_cores):
    key = ("prog", n_cores)
    if key not in _CACHE:
        nc = bacc.Bacc("TRN2", target_bir_lowering=False, debug=False,
                       num_devices=n_cores)
        build(nc, T=4096)
        nc.compile()
        _CACHE[key] = nc
    return _CACHE[key]


def kernel(x, time_decay, time_first, time_mix_k, time_mix_v, time_mix_r,
           Wk, Wv, Wr, Wo):
    """WKV attention: x [8, 4096, 1024] fp32 -> out [8, 4096, 1024] fp32.

    Shards batch across the 8 NeuronCores (one batch element per core).
    """
    global _last_exec_time_ns
    import os
    from concourse import bass_utils

    x = np.asarray(x, dtype=np.float32)
    B = x.shape[0]
    base = pack_inputs(x[0], np.asarray(time_decay), np.asarray(time_first),
                       np.asarray(time_mix_k), np.asarray(time_mix_v),
                       np.asarray(time_mix_r), np.asarray(Wk), np.asarray(Wv),
                       np.asarray(Wr), np.asarray(Wo))
    import ml_dtypes
    bf16 = ml_dtypes.bfloat16
    in_maps = []
    for b in range(B):
        m = dict(base)
        xt = np.zeros((D, x.shape[1] + 1), dtype=bf16)
        xt[:, 1:] = x[b].T.astype(bf16)
        m["xt"] = np.ascontiguousarray(xt)
        in_maps.append(m)

    nc = _get_program(B)
    trace = os.environ.get("WKV_TRACE", "0") == "1"
    r = bass_utils.run_bass_kernel_spmd(nc, in_maps, core_ids=list(range(B)),
                                        trace=trace)
    _last_exec_time_ns = r.exec_time_ns
    return np.stack([r.results[b]["o"] for b in range(B)]).astype(np.float32)
